# revision 53
# baseline (speedup 1.0000x reference)
"""Trainium2 Bass kernel for nn_AnalogicalReasoning (gnn_message_passing).

Data-parallel over batch B=8 across the 8 NeuronCores (one batch element per
core). Per core everything is fused on-chip:

  - relation encoder for src/tgt computed in [hidden, pair] layout so that
    LayerNorm reductions over the hidden axis become ones-vector matmuls on
    the TensorEngine (partition-axis reductions are impossible on DVE).
  - the rel-encoder output matmul is fused into the mapping projection: the
    relation mask scales whole relT columns, so it commutes through map_W1
    and P0 = mask o (comb^T Xn + bias) with comb = rel_W2 @ map_W1-half
    computed once on-device (relT is never materialized).
  - the mapping network's LayerNorm over cat([src_rel_s, tgt_rel_t]) @ W1 is
    algebraically separated: x[s,t,:] = P[:,s] + Q[:,t], so mean/var split
    into per-s and per-t statistics plus a cross term (2/H)*Pc^T@Qc computed
    as one matmul per block. With map_be == 0 (checked on host),
       relu(LN(x)) @ w2 = rstd[s,t] * (relu(Pg[:,s] + Qg[:,t]) @ w2)
    which moves the rstd multiply out of the O(S*T*H) inner loop entirely.
    rstd = reciprocal_approx_fast(sqrt(var+eps)) keeps ACT on one table set.
  - the big pass builds Z = max(Qg + p_s, 0) with dual-op tensor_scalar ops
    on DVE (bf16, 2x mode; the per-partition AP scalar caps the mode) with
    every third op routed to ScalarE relu to load-balance. Each matmul packs
    two s-values (N=512) with w2 replicated to M=32 columns; quads of four
    matmuls run concurrently in distinct PE col groups (psum partitions
    {0,32,64,96}), are drained partition-preserving by DVE/ACT into sparse
    SBUF, and scattered to dense score tiles by SBUF->SBUF DMAs.
  - a ~4us PE warmup spin unthrottles the HAM clock gate before the
    latency-bound encoder phase; map_b2/cor_b2 drop (softmax shift
    invariance); softmax needs no max-shift (scores are O(1) by
    construction).

Output per core: [257, 256] f32 — rows 0..255 mapping softmax, row 256 the
flattened 16x16 correspondence softmax.
"""

import numpy as np

B, N, E, H = 8, 16, 256, 256
S = N * N          # 256 source/target relation pairs
EPS = 1e-5
NCORES = 8
NB = S // 128      # 2 s-blocks of 128 per core
ACT_EVERY = 4      # route every 4th big-pass Z-op to ScalarE

_COMPILED = None


# ---------------------------------------------------------------------------
# graph construction
# ---------------------------------------------------------------------------

def _emit(nc, tc, io):
    import os
    import concourse.mybir as mybir
    STAGE = int(os.environ.get("K_STAGE", "9"))

    f32 = mybir.dt.float32
    bf16 = mybir.dt.bfloat16
    AF = mybir.ActivationFunctionType
    ALU = mybir.AluOpType

    out_d = io["out"]

    from contextlib import ExitStack
    with tc.tile_pool(name="wp", bufs=1) as wp, \
         tc.tile_pool(name="zp", bufs=24) as zp:

        ones_col = wp.tile([128, 1], f32)
        nc.vector.memset(ones_col, 1.0)

        # ---- weights into SBUF, chunked so partition = contraction index ----
        def load_mat(name, rows):
            t = wp.tile([128, rows // 128, io[name].shape[1]], f32, name=f"{name}_sb")
            nc.sync.dma_start(t, io[name].rearrange("(c p) h -> p c h", p=128))
            return t

        def load_vec(name):
            t = wp.tile([128, 2], f32, name=f"{name}_sb")
            nc.sync.dma_start(t, io[name].rearrange("(c p) -> p c", p=128))
            return t

        relW1_sb = load_mat("rel_W1", 2 * E)     # [128, 4, 256]
        relW2_sb = load_mat("rel_W2", H)         # [128, 2, 256]
        mapW1_sb = load_mat("map_W1", 2 * H)     # [128, 4, 256]
        corW1_sb = load_mat("cor_W1", 2 * E)     # [128, 4, 256]
        relb1_sb = load_vec("rel_b1")
        relg1_sb = load_vec("rel_g1")
        relbe1_sb = load_vec("rel_be1")
        relb2_sb = load_vec("rel_b2")
        mapb1_sb = load_vec("map_b1")
        mapg_sb = load_vec("map_g")
        corb1_sb = load_vec("cor_b1")

        mapW2_sb = wp.tile([128, 2], f32)
        nc.sync.dma_start(mapW2_sb, io["map_W2"].rearrange("(c p) o -> p (c o)", p=128))
        w2bf = wp.tile([128, 2], bf16)
        nc.vector.tensor_copy(w2bf, mapW2_sb)
        # w2 replicated to 32 columns: big-pass matmuls write all 32 rows of
        # a col group (M=32) so psum tiles are fully initialized before drain
        w2rep = wp.tile([128, 2, 32], bf16)
        for c in range(2):
            nc.vector.tensor_copy(w2rep[:, c, :],
                                  w2bf[:, c:c + 1].broadcast_to((128, 32)))
        corW2_sb = wp.tile([128, 2], f32)
        nc.sync.dma_start(corW2_sb, io["cor_W2"].rearrange("(c p) o -> p (c o)", p=128))

        # entities transposed [e, i] (2 e-chunks); small strided DMA
        entT = {}
        for side, name in (("s", "source_entities"), ("t", "target_entities")):
            t = wp.tile([128, 2, N], f32, name=f"entT_{side}")
            src = io[name].rearrange("i (c p) -> p c i", p=128)
            for c in range(2):
                nc.sync.dma_start(t[:, c, :], src[:, c, :])
            entT[side] = t

        # relation masks as [1, 256] rows: 1.0 where rel > 0
        mask = {}
        for side, name in (("s", "source_relations"), ("t", "target_relations")):
            raw = wp.tile([1, S], f32, name=f"relraw_{side}")
            nc.sync.dma_start(raw, io[name].rearrange("i j -> (i j)").unsqueeze(0))
            m = wp.tile([1, S], f32, name=f"mask_{side}")
            nc.vector.tensor_scalar(m, raw, 0.0, None, op0=ALU.is_gt)
            mask[side] = m

        ones_row = wp.tile([1, 128], f32)
        nc.vector.memset(ones_row, 1.0)
        twos_row = wp.tile([1, 128], f32)
        nc.vector.memset(twos_row, 2.0)
        eps_col = wp.tile([128, 1], f32)
        nc.vector.memset(eps_col, EPS)

        # PE warmup: ~4us of dense dummy matmuls while input DMAs run, so
        # the HAM clock gate reaches 2.4 GHz before the first real matmul
        # (otherwise the dependency-latency-bound encoder phase runs at
        # 1.2 GHz throughout).
        warm_in = wp.tile([128, 512], bf16)
        nc.vector.memset(warm_in, 0.0)
        warm_w = wp.tile([128, 1], bf16)
        nc.vector.memset(warm_w, 0.0)
        with tc.tile_pool(name="warmp", bufs=1, space="PSUM") as warmp:
            warm_ps = warmp.tile([1, 2 * S], f32)
            for _ in range(16):
                nc.tensor.matmul(warm_ps, warm_w, warm_in, start=True, stop=True)

        # encoder-phase PSUM pool (released before the big pass to stay
        # within the 8 PSUM banks)
        enc_ctx = ExitStack()
        pp = enc_ctx.enter_context(tc.tile_pool(name="pp", bufs=2, space="PSUM"))

        def bcast(pool, row_sb, n, tag="bcast", rows=None):
            # replicate a [1, n] SBUF row across all 128 psum partitions,
            # scaled by the value in `rows` (default ones)
            bp = pool.tile([128, n], f32, name="bc_ps", tag=tag,
                           bufs=2 if tag == "bcast" else None)
            nc.tensor.matmul(bp, rows if rows is not None else ones_row,
                             row_sb, start=True, stop=True)
            return bp

        # ---------------------------------------------------------------
        # relation encoder (per side) -> relT chunks [128, 256] x2 (k, pair)
        # ---------------------------------------------------------------
        def encode(side):
            eT = entT[side]
            # AT/CT: [h, i] = Wi^T @ entT ; Wj rows are chunks 2,3 of rel_W1
            AT, CT = [], []
            for mb in range(2):
                at_ps = pp.tile([128, N], f32, name="at_ps", tag="enc_ps")
                for c in range(2):
                    nc.tensor.matmul(at_ps, relW1_sb[:, c, mb * 128:(mb + 1) * 128],
                                     eT[:, c, :], start=(c == 0), stop=(c == 1))
                a = wp.tile([128, N], f32, name=f"AT{side}{mb}")
                nc.scalar.copy(a, at_ps)
                AT.append(a)
                ct_ps = pp.tile([128, N], f32, name="ct_ps", tag="enc_ps")
                for c in range(2):
                    nc.tensor.matmul(ct_ps, relW1_sb[:, 2 + c, mb * 128:(mb + 1) * 128],
                                     eT[:, c, :], start=(c == 0), stop=(c == 1))
                cc = wp.tile([128, N], f32, name=f"CT{side}{mb}")
                nc.scalar.activation(cc, ct_ps, AF.Identity,
                                     bias=relb1_sb[:, mb:mb + 1])
                CT.append(cc)

            # X[h, i*16+j] = AT[h,i] + CT[h,j]
            X = []
            for mb in range(2):
                x = wp.tile([128, S], f32, name=f"X{side}{mb}")
                nc.vector.tensor_tensor(
                    x.rearrange("p (i j) -> p i j", i=N),
                    CT[mb].unsqueeze(1).broadcast_to((128, N, N)),
                    AT[mb].unsqueeze(2).broadcast_to((128, N, N)),
                    op=ALU.add)
                X.append(x)

            # LayerNorm over h (partition axis, via ones-matmuls)
            sum_ps = pp.tile([1, S], f32, name="sum_ps", tag="enc_row", bufs=1)
            for c in range(2):
                nc.tensor.matmul(sum_ps, ones_col, X[c], start=(c == 0), stop=(c == 1))
            mean_row = wp.tile([1, S], f32, name=f"mean_{side}")
            nc.scalar.mul(mean_row, sum_ps, 1.0 / H)
            mean_bc = bcast(pp, mean_row, S)
            usq = []
            for c in range(2):
                nc.vector.tensor_tensor(X[c], X[c], mean_bc, op=ALU.subtract)
                u = wp.tile([128, S], f32, name=f"usq{side}{c}")
                nc.vector.tensor_mul(u, X[c], X[c])
                usq.append(u)
            var_ps = pp.tile([1, S], f32, name="var_ps", tag="enc_row", bufs=1)
            for c in range(2):
                nc.tensor.matmul(var_ps, ones_col, usq[c], start=(c == 0), stop=(c == 1))
            sqrt_row = wp.tile([1, S], f32, name=f"sqrtr_{side}")
            nc.scalar.activation(sqrt_row, var_ps, AF.Sqrt,
                                 bias=eps_col[0:1, :], scale=1.0 / H)
            rstd_row = wp.tile([1, S], f32, name=f"rstdr_{side}")
            nc.vector.reciprocal_approx_fast(rstd_row, sqrt_row)
            rstd_bc = bcast(pp, rstd_row, S)
            for c in range(2):
                nc.vector.tensor_tensor(X[c], X[c], rstd_bc, op=ALU.mult)
                nc.scalar.activation(X[c], X[c], AF.Relu,
                                     bias=relbe1_sb[:, c:c + 1],
                                     scale=relg1_sb[:, c:c + 1])

            return X

        # ---------------------------------------------------------------
        # fuse rel-encoder output matmul with the mapping projection:
        # the relation mask scales whole relT columns, so it commutes
        # through map_W1:
        #   P0 = map_Wa^T (mask o (rel_W2^T Xn + b2))
        #      = (mask o (comb_a^T Xn + bias_a))  with comb_a = rel_W2 map_Wa
        # Computed once on-device (transpose rel_W2 via PE, 2 matmuls).
        # ---------------------------------------------------------------
        from concourse import masks
        ident = wp.tile([128, 128], f32)
        masks.make_identity(nc, ident)
        relW2T = wp.tile([128, 2, 256], f32)     # [k-part, k-chunk a, h]
        for a in range(2):
            for c in range(2):
                tp_ps = pp.tile([128, 128], f32, name="tp_ps", tag="enc_ops")
                nc.tensor.transpose(tp_ps, relW2_sb[:, c, a * 128:(a + 1) * 128],
                                    ident)
                nc.vector.tensor_copy(relW2T[:, a, c * 128:(c + 1) * 128], tp_ps)
        comb = {}
        biasc = {}
        for which, wch in (("a", 0), ("b", 2)):
            cw = wp.tile([128, 2, 256], f32, name=f"comb{which}")  # [h-part, hb, h']
            for hb in range(2):
                ps = pp.tile([128, S], f32, name="comb_ps", tag="enc_ops")
                for kc in range(2):
                    nc.tensor.matmul(ps, relW2T[:, kc, hb * 128:(hb + 1) * 128],
                                     mapW1_sb[:, wch + kc, :],
                                     start=(kc == 0), stop=(kc == 1))
                nc.vector.tensor_copy(cw[:, hb, :], ps)
            comb[which] = cw
            bc_t = wp.tile([128, 2], f32, name=f"biasc{which}")    # [h'-part, hb]
            for hb in range(2):
                ps1 = pp.tile([128, 1], f32, name="bc1_ps", tag="enc_col", bufs=1)
                for kc in range(2):
                    nc.tensor.matmul(ps1, mapW1_sb[:, wch + kc, hb * 128:(hb + 1) * 128],
                                     relb2_sb[:, kc:kc + 1],
                                     start=(kc == 0), stop=(kc == 1))
                nc.vector.tensor_copy(bc_t[:, hb:hb + 1], ps1)
            biasc[which] = bc_t

        Xs = encode("s")
        Xt = encode("t")

        def project(Xn, which, side, b1_col=None):
            mask_bc = bcast(pp, mask[side], S)
            out = []
            for hb in range(2):
                ps = pp.tile([128, S], f32, name="pq_ps", tag="enc_ops")
                for c in range(2):
                    nc.tensor.matmul(ps, comb[which][:, c, hb * 128:(hb + 1) * 128],
                                     Xn[c], start=(c == 0), stop=(c == 1))
                o = wp.tile([128, S], f32, name=f"proj{which}{hb}")
                nc.scalar.activation(o, ps, AF.Identity,
                                     bias=biasc[which][:, hb:hb + 1])
                nc.vector.tensor_tensor(o, o, mask_bc, op=ALU.mult)
                if b1_col is not None:
                    nc.vector.tensor_scalar(o, o, b1_col[hb], None, op0=ALU.add)
                out.append(o)
            return out

        P = project(Xs, "a", "s")
        Q = project(Xt, "b", "t",
                    [mapb1_sb[:, 0:1], mapb1_sb[:, 1:2]])

        def center_stats(Xc, label):
            sum_ps = pp.tile([1, S], f32, name="msum_ps", tag="enc_row", bufs=1)
            for c in range(2):
                nc.tensor.matmul(sum_ps, ones_col, Xc[c], start=(c == 0), stop=(c == 1))
            mrow = wp.tile([1, S], f32, name=f"mrow_{label}")
            nc.scalar.mul(mrow, sum_ps, 1.0 / H)
            m_bc = bcast(pp, mrow, S)
            usq = []
            for c in range(2):
                nc.vector.tensor_tensor(Xc[c], Xc[c], m_bc, op=ALU.subtract)
                u = wp.tile([128, S], f32, name=f"musq_{label}{c}")
                nc.vector.tensor_mul(u, Xc[c], Xc[c])
                usq.append(u)
            return usq

        usqP = center_stats(P, "P")   # P, Q centered in place now
        usqQ = center_stats(Q, "Q")

        varS_col = []
        for blk in range(NB):
            v_ps = pp.tile([128, 1], f32, name="vs_ps", tag="enc_col", bufs=1)
            for c in range(2):
                nc.tensor.matmul(v_ps, usqP[c][:, blk * 128:(blk + 1) * 128],
                                 ones_col, start=(c == 0), stop=(c == 1))
            v = wp.tile([128, 1], f32, name=f"varS{blk}")
            nc.scalar.mul(v, v_ps, 1.0 / H)
            varS_col.append(v)

        vt_ps = pp.tile([1, S], f32, name="vt_ps", tag="enc_row", bufs=1)
        for c in range(2):
            nc.tensor.matmul(vt_ps, ones_col, usqQ[c], start=(c == 0), stop=(c == 1))
        varT_row = wp.tile([1, S], f32)
        nc.scalar.mul(varT_row, vt_ps, 1.0 / H)

        Pg, Qg = [], []
        for c in range(2):
            pg = wp.tile([128, S], f32, name=f"Pg{c}")
            nc.vector.tensor_scalar(pg, P[c], mapg_sb[:, c:c + 1], None, op0=ALU.mult)
            Pg.append(pg)
            qg = wp.tile([128, S], bf16, name=f"Qg{c}")
            nc.vector.tensor_scalar(qg, Q[c], mapg_sb[:, c:c + 1], None, op0=ALU.mult)
            Qg.append(qg)

        # ---------------------------------------------------------------
        # per-block (128 s values): cross-term matmul -> rstd [128, 256],
        # then the big pass. Each matmul's moving tensor packs TWO s values
        # (N=512 = 2x256); matmul outputs must land on psum partitions
        # {0,32,64,96}, so duos are processed in quads (duo 4k+j -> partition
        # 32j of a quad psum tile; the 4 matmuls run concurrently in distinct
        # PE col groups). Quads are drained partition-preserving by DVE/ACT
        # into a sparse SBUF tile, then an SBUF->SBUF DMA scatters rows into
        # the dense [128, 256] score tile:
        #   s_local = 8*k + 2*j + u   (k quad, j col group, u duo half)
        # ---------------------------------------------------------------
        enc_ctx.close()
        scp_ctx = ExitStack()
        scp = scp_ctx.enter_context(tc.tile_pool(name="scp", bufs=2, space="PSUM"))
        act_ctr = 0
        drain_ctr = 0
        rstd_ts = []
        for blk in range(NB):
            dot_ps = scp.tile([128, S], f32, name="dot_ps", tag="dot")
            for c in range(2):
                nc.tensor.matmul(dot_ps, P[c][:, blk * 128:(blk + 1) * 128],
                                 Q[c], start=(c == 0), stop=(c == 1))
            var_t = wp.tile([128, S], f32, name=f"var_t{blk}", tag="var_t", bufs=2)
            nc.vector.tensor_scalar(var_t, dot_ps, 2.0 / H, varS_col[blk],
                                    op0=ALU.mult, op1=ALU.add)
            vt_bc = bcast(scp, varT_row, S, tag="dot")
            nc.vector.tensor_tensor(var_t, var_t, vt_bc, op=ALU.add)
            sq_t = wp.tile([128, S], f32, name=f"sq_t{blk}", tag="sq_t", bufs=2)
            nc.scalar.activation(sq_t, var_t, AF.Sqrt, bias=eps_col)
            rstd_t = wp.tile([128, S], f32, name=f"rstd_t{blk}", tag="rstd_t", bufs=2)
            nc.vector.reciprocal_approx_fast(rstd_t, sq_t)
            rstd_ts.append(rstd_t)

        for blk in range(NB):
            rstd_t = rstd_ts[blk]
            if STAGE < 2:
                nc.sync.dma_start(out_d[blk * 128:(blk + 1) * 128, :], rstd_t)
                continue

            ssb = wp.tile([128, S], f32, name=f"ssb{blk}", tag="ssb", bufs=2)
            for g in range(8):              # 8 groups of 2 quads
                # 2 quads share one 2-bank psum tensor; quad qq fills free
                # half qq. Drained with a single [4, 1024] copy.
                qgps = scp.tile([128, 2, 2, S], f32, name="qgps", tag="quad",
                                bufs=3)
                sparse = wp.tile([128, 2, 2, S], f32, name="sparse",
                                 tag="sparse", bufs=3)
                for qq in range(2):
                    k = 2 * g + qq
                    for c in range(2):
                        zs = []
                        for j in range(4):
                            z = zp.tile([128, 2 * S], bf16, name="z", tag="z")
                            for u in range(2):
                                col = blk * 128 + 8 * k + 2 * j + u
                                zslice = z[:, u * S:(u + 1) * S]
                                if act_ctr % ACT_EVERY == ACT_EVERY - 1:
                                    nc.scalar.activation(zslice, Qg[c], AF.Relu,
                                                         bias=Pg[c][:, col:col + 1])
                                else:
                                    nc.vector.tensor_scalar(
                                        zslice, Qg[c], Pg[c][:, col:col + 1], 0.0,
                                        op0=ALU.add, op1=ALU.max)
                                act_ctr += 1
                            zs.append(z)
                        for j in range(4):
                            nc.tensor.matmul(
                                qgps[32 * j:32 * j + 32, qq, :, :],
                                w2rep[:, c, :], zs[j],
                                start=(c == 0), stop=(c == 1),
                                tile_position=(0, 32 * j),
                                skip_group_check=True)
                # partition-preserving drain of both quads. Only rows {32j}
                # are valid, but compute APs need partition step 1 and DVE
                # cost is free-dim-bound, so copy all 128 rows densely.
                drain_in = qgps.rearrange("p q u t -> p (q u t)")
                drain_out = sparse.rearrange("p q u t -> p (q u t)")
                if drain_ctr % 2 == 0:
                    nc.vector.tensor_copy(drain_out, drain_in)
                else:
                    nc.scalar.copy(drain_out, drain_in)
                drain_ctr += 1
                if STAGE < 3:
                    continue
                # scatter: rows {32j} -> dense ssb rows 8k+2j+u, per (quad, u)
                # rows 8k+2j+u iterate consecutively in (j, u) order, so one
                # DMA per quad writes the contiguous range [8k, 8k+8)
                sp_v = sparse.rearrange("(j x) q u t -> j x q u t", j=4)
                for qq in range(2):
                    k = 2 * g + qq
                    nc.sync.dma_start(ssb[8 * k:8 * k + 8, :],
                                      sp_v[:, 0, qq, :, :])

            if STAGE < 3:
                nc.sync.dma_start(out_d[blk * 128:(blk + 1) * 128, :],
                                  sparse[:, 0, 0, :])
                continue

            # epilogue: rstd multiply + softmax over t (no max-shift needed:
            # |scores| is small by construction, exp cannot overflow)
            nc.vector.tensor_tensor(ssb, ssb, rstd_t, op=ALU.mult)
            omap = wp.tile([128, S], f32, name=f"omap{blk}", tag="omap", bufs=2)
            sume = wp.tile([128, 1], f32, name="sume", tag="smx2", bufs=4)
            nc.scalar.activation(omap, ssb, AF.Exp, accum_out=sume)
            rec = wp.tile([128, 1], f32, name="rec", tag="smx3", bufs=4)
            nc.vector.reciprocal(rec, sume)
            nc.vector.tensor_scalar(omap, omap, rec, None, op0=ALU.mult)
            nc.sync.dma_start(out_d[blk * 128:(blk + 1) * 128, :], omap)

        scp_ctx.close()

        if STAGE < 4:
            return

        # ---------------------------------------------------------------
        # correspondence scorer (tiny): relu(Pe_i + Qe_j) @ w2, softmax
        # ---------------------------------------------------------------
        cp_ctx = ExitStack()
        cp = cp_ctx.enter_context(tc.tile_pool(name="cp", bufs=2, space="PSUM"))
        PeT, QeT = [], []
        for mb in range(2):
            pe_ps = cp.tile([128, N], f32, name="pe_ps", tag="cor_ps")
            for c in range(2):
                nc.tensor.matmul(pe_ps, corW1_sb[:, c, mb * 128:(mb + 1) * 128],
                                 entT["s"][:, c, :], start=(c == 0), stop=(c == 1))
            pe = wp.tile([128, N], f32, name=f"PeT{mb}")
            nc.scalar.copy(pe, pe_ps)
            PeT.append(pe)
            qe_ps = cp.tile([128, N], f32, name="qe_ps", tag="cor_ps")
            for c in range(2):
                nc.tensor.matmul(qe_ps, corW1_sb[:, 2 + c, mb * 128:(mb + 1) * 128],
                                 entT["t"][:, c, :], start=(c == 0), stop=(c == 1))
            qe = wp.tile([128, N], f32, name=f"QeT{mb}")
            nc.scalar.activation(qe, qe_ps, AF.Identity, bias=corb1_sb[:, mb:mb + 1])
            QeT.append(qe)

        cs_ps = cp.tile([1, S], f32, name="cs_ps", tag="cor_row", bufs=1)
        for mb in range(2):
            xr = wp.tile([128, S], f32, name=f"corX{mb}")
            nc.vector.tensor_tensor(
                xr.rearrange("p (i j) -> p i j", i=N),
                QeT[mb].unsqueeze(1).broadcast_to((128, N, N)),
                PeT[mb].unsqueeze(2).broadcast_to((128, N, N)),
                op=ALU.add)
            nc.vector.tensor_scalar(xr, xr, 0.0, None, op0=ALU.max)
            nc.tensor.matmul(cs_ps, corW2_sb[:, mb:mb + 1], xr,
                             start=(mb == 0), stop=(mb == 1))

        cs_row = wp.tile([1, S], f32)
        nc.scalar.copy(cs_row, cs_ps)
        if os.environ.get("K_COR_DEBUG"):
            nc.sync.dma_start(out_d[S:S + 1, :], cs_row)
            cp_ctx.close()
            return
        # bounce through DRAM: partition-expanding SBUF->SBUF DMAs are not
        # expressible as a single regular AP
        dr_ctx = ExitStack()
        drp = dr_ctx.enter_context(tc.tile_pool(name="drp", bufs=1, space="DRAM"))
        cs_dram = drp.tile([1, S], f32, name="cs_dram")
        nc.sync.dma_start(cs_dram, cs_row)
        corr2d = wp.tile([N, N], f32)
        nc.sync.dma_start(corr2d, cs_dram.rearrange("o (i j) -> (o i) j", i=N))
        if os.environ.get("K_COR_DEBUG2"):
            nc.sync.dma_start(out_d[S:S + 1, :].rearrange("o (i j) -> (o i) j", i=N), corr2d)
            cp_ctx.close()
            dr_ctx.close()
            return
        cmax = wp.tile([N, 1], f32)
        nc.vector.tensor_reduce(cmax, corr2d, axis=mybir.AxisListType.X,
                                op=ALU.max, negate=True)
        cex = wp.tile([N, N], f32)
        csum = wp.tile([N, 1], f32)
        nc.scalar.activation(cex, corr2d, AF.Exp, bias=cmax, accum_out=csum)
        crec = wp.tile([N, 1], f32)
        nc.vector.reciprocal(crec, csum)
        nc.vector.tensor_scalar(cex, cex, crec, None, op0=ALU.mult)
        nc.sync.dma_start(out_d[S:S + 1, :].rearrange("o (i j) -> (o i) j", i=N), cex)
        dr_ctx.close()
        cp_ctx.close()


def _declare_io(nc):
    import concourse.mybir as mybir
    f32 = mybir.dt.float32
    shapes = {
        "source_entities": [N, E], "source_relations": [N, N],
        "target_entities": [N, E], "target_relations": [N, N],
        "rel_W1": [2 * E, H], "rel_b1": [H], "rel_g1": [H], "rel_be1": [H],
        "rel_W2": [H, H], "rel_b2": [H],
        "map_W1": [2 * H, H], "map_b1": [H], "map_g": [H],
        "map_W2": [H, 1],
        "cor_W1": [2 * E, H], "cor_b1": [H], "cor_W2": [H, 1],
    }
    io = {k: nc.dram_tensor(k, v, f32, kind="ExternalInput").ap()
          for k, v in shapes.items()}
    io["out"] = nc.dram_tensor("out", [S + 1, S], f32, kind="ExternalOutput").ap()
    return io


def _build():
    import concourse.bacc as bacc
    import concourse.mybir as mybir
    from concourse.tile import TileContext
    nc = bacc.Bacc("TRN2", target_bir_lowering=False, debug=False,
                   enable_asserts=False)
    io = _declare_io(nc)
    with TileContext(nc) as tc:
        _emit(nc, tc, io)
    nc.finalize()
    return nc


def _get_compiled():
    global _COMPILED
    if _COMPILED is None:
        _COMPILED = _build()
    return _COMPILED


def _make_in_maps(inputs):
    names = ["source_entities", "source_relations", "target_entities",
             "target_relations"]
    weights = ["rel_W1", "rel_b1", "rel_g1", "rel_be1", "rel_W2", "rel_b2",
               "map_W1", "map_b1", "map_g", "map_W2", "cor_W1", "cor_b1",
               "cor_W2"]
    wmap = {k: np.ascontiguousarray(np.asarray(inputs[k], np.float32))
            for k in weights}
    in_maps = []
    for i in range(NCORES):
        m = dict(wmap)
        for k in names:
            m[k] = np.ascontiguousarray(np.asarray(inputs[k], np.float32)[i])
        in_maps.append(m)
    return in_maps


def _run_device(inputs, trace=False):
    from concourse.bass_utils import run_bass_kernel_spmd
    nc = _get_compiled()
    res = run_bass_kernel_spmd(nc, _make_in_maps(inputs),
                               core_ids=list(range(NCORES)), trace=trace)
    outs = [np.asarray(r["out"], np.float32) for r in res.results]
    maps = np.stack([o[:S, :] for o in outs]).reshape(B, S, S)
    cors = np.stack([o[S].reshape(N, N) for o in outs])
    return (maps, cors), res


# ---------------------------------------------------------------------------
# numpy fallback (general path; only used if fast-path assumptions break)
# ---------------------------------------------------------------------------

def _ln_np(x, g, b):
    m = x.mean(-1, keepdims=True)
    v = ((x - m) ** 2).mean(-1, keepdims=True)
    return (x - m) / np.sqrt(v + EPS) * g + b


def _softmax_np(x):
    e = np.exp(x - x.max(-1, keepdims=True))
    return e / e.sum(-1, keepdims=True)


def _reference_np(inputs):
    i = {k: np.asarray(v, np.float64) for k, v in inputs.items()}
    n, e = i["source_entities"].shape[1], i["source_entities"].shape[2]

    def enc(ent, rel):
        Wi, Wj = i["rel_W1"][:e], i["rel_W1"][e:]
        h = (np.einsum("bie,eh->bih", ent, Wi)[:, :, None, :]
             + np.einsum("bje,eh->bjh", ent, Wj)[:, None, :, :] + i["rel_b1"])
        h = np.maximum(_ln_np(h, i["rel_g1"], i["rel_be1"]), 0.0)
        h = np.einsum("bijh,hk->bijk", h, i["rel_W2"]) + i["rel_b2"]
        h = h * (rel > 0)[..., None]
        return h.reshape(ent.shape[0], n * n, -1)

    def pair(a, b, W1, b1, W2, b2, g=None, be=None):
        d = a.shape[-1]
        h = (np.einsum("bsd,dh->bsh", a, W1[:d])[:, :, None, :]
             + np.einsum("btd,dh->bth", b, W1[d:])[:, None, :, :] + b1)
        if g is not None:
            h = _ln_np(h, g, be)
        h = np.maximum(h, 0.0)
        return (np.einsum("bsth,ho->bsto", h, W2) + b2)[..., 0]

    sr = enc(i["source_entities"], i["source_relations"])
    tr = enc(i["target_entities"], i["target_relations"])
    ms = pair(sr, tr, i["map_W1"], i["map_b1"], i["map_W2"],
              i.get("map_b2", np.zeros(1)), i["map_g"], i["map_be"])
    cs = pair(i["source_entities"], i["target_entities"], i["cor_W1"],
              i["cor_b1"], i["cor_W2"], i.get("cor_b2", np.zeros(1)))
    return (_softmax_np(ms).astype(np.float32), _softmax_np(cs).astype(np.float32))


def _fast_path_ok(inputs):
    try:
        if tuple(np.asarray(inputs["source_entities"]).shape) != (B, N, E):
            return False
        if not np.all(np.asarray(inputs["map_be"]) == 0.0):
            return False
        for k in ("rel_W1", "rel_W2", "map_W1", "map_W2", "cor_W1", "cor_W2"):
            _ = inputs[k]
        return True
    except Exception:
        return False


def kernel(**inputs):
    if not _fast_path_ok(inputs):
        return _reference_np(inputs)
    (maps, cors), _ = _run_device(inputs)
    return maps, cors


if __name__ == "__main__":
    import reference as ref
    inp = ref.setup_inputs()
    m, c = kernel(**inp)
    print("map", m.shape, m.dtype, "cor", c.shape, c.dtype)


# revision 54
# speedup vs baseline: 1.0076x; 1.0076x over previous
"""Trainium2 Bass kernel for nn_AnalogicalReasoning (gnn_message_passing).

Data-parallel over batch B=8 across the 8 NeuronCores (one batch element per
core). Per core everything is fused on-chip:

  - relation encoder for src/tgt computed in [hidden, pair] layout so that
    LayerNorm reductions over the hidden axis become ones-vector matmuls on
    the TensorEngine (partition-axis reductions are impossible on DVE).
  - the rel-encoder output matmul is fused into the mapping projection: the
    relation mask scales whole relT columns, so it commutes through map_W1
    and P0 = mask o (comb^T Xn + bias) with comb = rel_W2 @ map_W1-half
    computed once on-device (relT is never materialized).
  - the mapping network's LayerNorm over cat([src_rel_s, tgt_rel_t]) @ W1 is
    algebraically separated: x[s,t,:] = P[:,s] + Q[:,t], so mean/var split
    into per-s and per-t statistics plus a cross term (2/H)*Pc^T@Qc computed
    as one matmul per block. With map_be == 0 (checked on host),
       relu(LN(x)) @ w2 = rstd[s,t] * (relu(Pg[:,s] + Qg[:,t]) @ w2)
    which moves the rstd multiply out of the O(S*T*H) inner loop entirely.
    rstd = reciprocal_approx_fast(sqrt(var+eps)) keeps ACT on one table set.
  - the big pass builds Z = max(Qg + p_s, 0) with dual-op tensor_scalar ops
    on DVE (bf16, 2x mode; the per-partition AP scalar caps the mode) with
    every third op routed to ScalarE relu to load-balance. Each matmul packs
    two s-values (N=512) with w2 replicated to M=32 columns; quads of four
    matmuls run concurrently in distinct PE col groups (psum partitions
    {0,32,64,96}), are drained partition-preserving by DVE/ACT into sparse
    SBUF, and scattered to dense score tiles by SBUF->SBUF DMAs.
  - a ~4us PE warmup spin unthrottles the HAM clock gate before the
    latency-bound encoder phase; map_b2/cor_b2 drop (softmax shift
    invariance); softmax needs no max-shift (scores are O(1) by
    construction).

Output per core: [257, 256] f32 — rows 0..255 mapping softmax, row 256 the
flattened 16x16 correspondence softmax.
"""

import numpy as np

B, N, E, H = 8, 16, 256, 256
S = N * N          # 256 source/target relation pairs
EPS = 1e-5
NCORES = 8
NB = S // 128      # 2 s-blocks of 128 per core
ACT_EVERY = 3      # route every 3rd big-pass Z-op to ScalarE

_COMPILED = None


# ---------------------------------------------------------------------------
# graph construction
# ---------------------------------------------------------------------------

def _emit(nc, tc, io):
    import os
    import concourse.mybir as mybir
    STAGE = int(os.environ.get("K_STAGE", "9"))

    f32 = mybir.dt.float32
    bf16 = mybir.dt.bfloat16
    AF = mybir.ActivationFunctionType
    ALU = mybir.AluOpType

    out_d = io["out"]

    from contextlib import ExitStack
    with tc.tile_pool(name="wp", bufs=1) as wp, \
         tc.tile_pool(name="zp", bufs=24) as zp:

        ones_col = wp.tile([128, 1], f32)
        nc.vector.memset(ones_col, 1.0)

        # ---- weights into SBUF, chunked so partition = contraction index ----
        def load_mat(name, rows):
            t = wp.tile([128, rows // 128, io[name].shape[1]], f32, name=f"{name}_sb")
            nc.sync.dma_start(t, io[name].rearrange("(c p) h -> p c h", p=128))
            return t

        def load_vec(name):
            t = wp.tile([128, 2], f32, name=f"{name}_sb")
            nc.sync.dma_start(t, io[name].rearrange("(c p) -> p c", p=128))
            return t

        relW1_sb = load_mat("rel_W1", 2 * E)     # [128, 4, 256]
        relW2_sb = load_mat("rel_W2", H)         # [128, 2, 256]
        mapW1_sb = load_mat("map_W1", 2 * H)     # [128, 4, 256]
        corW1_sb = load_mat("cor_W1", 2 * E)     # [128, 4, 256]
        relb1_sb = load_vec("rel_b1")
        relg1_sb = load_vec("rel_g1")
        relbe1_sb = load_vec("rel_be1")
        relb2_sb = load_vec("rel_b2")
        mapb1_sb = load_vec("map_b1")
        mapg_sb = load_vec("map_g")
        corb1_sb = load_vec("cor_b1")

        mapW2_sb = wp.tile([128, 2], f32)
        nc.sync.dma_start(mapW2_sb, io["map_W2"].rearrange("(c p) o -> p (c o)", p=128))
        w2bf = wp.tile([128, 2], bf16)
        nc.vector.tensor_copy(w2bf, mapW2_sb)
        # w2 replicated to 32 columns: big-pass matmuls write all 32 rows of
        # a col group (M=32) so psum tiles are fully initialized before drain
        w2rep = wp.tile([128, 2, 32], bf16)
        for c in range(2):
            nc.vector.tensor_copy(w2rep[:, c, :],
                                  w2bf[:, c:c + 1].broadcast_to((128, 32)))
        corW2_sb = wp.tile([128, 2], f32)
        nc.sync.dma_start(corW2_sb, io["cor_W2"].rearrange("(c p) o -> p (c o)", p=128))

        # entities transposed [e, i] (2 e-chunks); small strided DMA
        entT = {}
        for side, name in (("s", "source_entities"), ("t", "target_entities")):
            t = wp.tile([128, 2, N], f32, name=f"entT_{side}")
            src = io[name].rearrange("i (c p) -> p c i", p=128)
            for c in range(2):
                nc.sync.dma_start(t[:, c, :], src[:, c, :])
            entT[side] = t

        # relation masks as [1, 256] rows: 1.0 where rel > 0
        mask = {}
        for side, name in (("s", "source_relations"), ("t", "target_relations")):
            raw = wp.tile([1, S], f32, name=f"relraw_{side}")
            nc.sync.dma_start(raw, io[name].rearrange("i j -> (i j)").unsqueeze(0))
            m = wp.tile([1, S], f32, name=f"mask_{side}")
            nc.vector.tensor_scalar(m, raw, 0.0, None, op0=ALU.is_gt)
            mask[side] = m

        ones_row = wp.tile([1, 128], f32)
        nc.vector.memset(ones_row, 1.0)
        twos_row = wp.tile([1, 128], f32)
        nc.vector.memset(twos_row, 2.0)
        eps_col = wp.tile([128, 1], f32)
        nc.vector.memset(eps_col, EPS)

        # PE warmup: ~4us of dense dummy matmuls while input DMAs run, so
        # the HAM clock gate reaches 2.4 GHz before the first real matmul
        # (otherwise the dependency-latency-bound encoder phase runs at
        # 1.2 GHz throughout).
        warm_in = wp.tile([128, 512], bf16)
        nc.vector.memset(warm_in, 0.0)
        warm_w = wp.tile([128, 1], bf16)
        nc.vector.memset(warm_w, 0.0)
        with tc.tile_pool(name="warmp", bufs=1, space="PSUM") as warmp:
            warm_ps = warmp.tile([1, 2 * S], f32)
            for _ in range(16):
                nc.tensor.matmul(warm_ps, warm_w, warm_in, start=True, stop=True)

        # encoder-phase PSUM pool (released before the big pass to stay
        # within the 8 PSUM banks)
        enc_ctx = ExitStack()
        pp = enc_ctx.enter_context(tc.tile_pool(name="pp", bufs=2, space="PSUM"))

        def bcast(pool, row_sb, n, tag="bcast", rows=None):
            # replicate a [1, n] SBUF row across all 128 psum partitions,
            # scaled by the value in `rows` (default ones)
            bp = pool.tile([128, n], f32, name="bc_ps", tag=tag,
                           bufs=2 if tag == "bcast" else None)
            nc.tensor.matmul(bp, rows if rows is not None else ones_row,
                             row_sb, start=True, stop=True)
            return bp

        # ---------------------------------------------------------------
        # relation encoder (per side) -> relT chunks [128, 256] x2 (k, pair)
        # ---------------------------------------------------------------
        def encode(side):
            eT = entT[side]
            # AT/CT: [h, i] = Wi^T @ entT ; Wj rows are chunks 2,3 of rel_W1
            AT, CT = [], []
            for mb in range(2):
                at_ps = pp.tile([128, N], f32, name="at_ps", tag="enc_ps")
                for c in range(2):
                    nc.tensor.matmul(at_ps, relW1_sb[:, c, mb * 128:(mb + 1) * 128],
                                     eT[:, c, :], start=(c == 0), stop=(c == 1))
                a = wp.tile([128, N], f32, name=f"AT{side}{mb}")
                nc.scalar.copy(a, at_ps)
                AT.append(a)
                ct_ps = pp.tile([128, N], f32, name="ct_ps", tag="enc_ps")
                for c in range(2):
                    nc.tensor.matmul(ct_ps, relW1_sb[:, 2 + c, mb * 128:(mb + 1) * 128],
                                     eT[:, c, :], start=(c == 0), stop=(c == 1))
                cc = wp.tile([128, N], f32, name=f"CT{side}{mb}")
                nc.scalar.activation(cc, ct_ps, AF.Identity,
                                     bias=relb1_sb[:, mb:mb + 1])
                CT.append(cc)

            # X[h, i*16+j] = AT[h,i] + CT[h,j]
            X = []
            for mb in range(2):
                x = wp.tile([128, S], f32, name=f"X{side}{mb}")
                nc.vector.tensor_tensor(
                    x.rearrange("p (i j) -> p i j", i=N),
                    CT[mb].unsqueeze(1).broadcast_to((128, N, N)),
                    AT[mb].unsqueeze(2).broadcast_to((128, N, N)),
                    op=ALU.add)
                X.append(x)

            # LayerNorm over h (partition axis, via ones-matmuls)
            sum_ps = pp.tile([1, S], f32, name="sum_ps", tag="enc_row", bufs=1)
            for c in range(2):
                nc.tensor.matmul(sum_ps, ones_col, X[c], start=(c == 0), stop=(c == 1))
            mean_row = wp.tile([1, S], f32, name=f"mean_{side}")
            nc.scalar.mul(mean_row, sum_ps, 1.0 / H)
            mean_bc = bcast(pp, mean_row, S)
            usq = []
            for c in range(2):
                nc.vector.tensor_tensor(X[c], X[c], mean_bc, op=ALU.subtract)
                u = wp.tile([128, S], f32, name=f"usq{side}{c}")
                nc.vector.tensor_mul(u, X[c], X[c])
                usq.append(u)
            var_ps = pp.tile([1, S], f32, name="var_ps", tag="enc_row", bufs=1)
            for c in range(2):
                nc.tensor.matmul(var_ps, ones_col, usq[c], start=(c == 0), stop=(c == 1))
            sqrt_row = wp.tile([1, S], f32, name=f"sqrtr_{side}")
            nc.scalar.activation(sqrt_row, var_ps, AF.Sqrt,
                                 bias=eps_col[0:1, :], scale=1.0 / H)
            rstd_row = wp.tile([1, S], f32, name=f"rstdr_{side}")
            nc.vector.reciprocal_approx_fast(rstd_row, sqrt_row)
            rstd_bc = bcast(pp, rstd_row, S)
            for c in range(2):
                nc.vector.tensor_tensor(X[c], X[c], rstd_bc, op=ALU.mult)
                nc.scalar.activation(X[c], X[c], AF.Relu,
                                     bias=relbe1_sb[:, c:c + 1],
                                     scale=relg1_sb[:, c:c + 1])

            return X

        # ---------------------------------------------------------------
        # fuse rel-encoder output matmul with the mapping projection:
        # the relation mask scales whole relT columns, so it commutes
        # through map_W1:
        #   P0 = map_Wa^T (mask o (rel_W2^T Xn + b2))
        #      = (mask o (comb_a^T Xn + bias_a))  with comb_a = rel_W2 map_Wa
        # Computed once on-device (transpose rel_W2 via PE, 2 matmuls).
        # ---------------------------------------------------------------
        from concourse import masks
        ident = wp.tile([128, 128], f32)
        masks.make_identity(nc, ident)
        relW2T = wp.tile([128, 2, 256], f32)     # [k-part, k-chunk a, h]
        for a in range(2):
            for c in range(2):
                tp_ps = pp.tile([128, 128], f32, name="tp_ps", tag="enc_ops")
                nc.tensor.transpose(tp_ps, relW2_sb[:, c, a * 128:(a + 1) * 128],
                                    ident)
                nc.vector.tensor_copy(relW2T[:, a, c * 128:(c + 1) * 128], tp_ps)
        comb = {}
        biasc = {}
        for which, wch in (("a", 0), ("b", 2)):
            cw = wp.tile([128, 2, 256], f32, name=f"comb{which}")  # [h-part, hb, h']
            for hb in range(2):
                ps = pp.tile([128, S], f32, name="comb_ps", tag="enc_ops")
                for kc in range(2):
                    nc.tensor.matmul(ps, relW2T[:, kc, hb * 128:(hb + 1) * 128],
                                     mapW1_sb[:, wch + kc, :],
                                     start=(kc == 0), stop=(kc == 1))
                nc.vector.tensor_copy(cw[:, hb, :], ps)
            comb[which] = cw
            bc_t = wp.tile([128, 2], f32, name=f"biasc{which}")    # [h'-part, hb]
            for hb in range(2):
                ps1 = pp.tile([128, 1], f32, name="bc1_ps", tag="enc_col", bufs=1)
                for kc in range(2):
                    nc.tensor.matmul(ps1, mapW1_sb[:, wch + kc, hb * 128:(hb + 1) * 128],
                                     relb2_sb[:, kc:kc + 1],
                                     start=(kc == 0), stop=(kc == 1))
                nc.vector.tensor_copy(bc_t[:, hb:hb + 1], ps1)
            biasc[which] = bc_t

        Xs = encode("s")
        Xt = encode("t")

        def project(Xn, which, side, b1_col=None):
            mask_bc = bcast(pp, mask[side], S)
            out = []
            for hb in range(2):
                ps = pp.tile([128, S], f32, name="pq_ps", tag="enc_ops")
                for c in range(2):
                    nc.tensor.matmul(ps, comb[which][:, c, hb * 128:(hb + 1) * 128],
                                     Xn[c], start=(c == 0), stop=(c == 1))
                o = wp.tile([128, S], f32, name=f"proj{which}{hb}")
                nc.scalar.activation(o, ps, AF.Identity,
                                     bias=biasc[which][:, hb:hb + 1])
                nc.vector.tensor_tensor(o, o, mask_bc, op=ALU.mult)
                if b1_col is not None:
                    nc.vector.tensor_scalar(o, o, b1_col[hb], None, op0=ALU.add)
                out.append(o)
            return out

        P = project(Xs, "a", "s")
        Q = project(Xt, "b", "t",
                    [mapb1_sb[:, 0:1], mapb1_sb[:, 1:2]])

        def center_stats(Xc, label):
            sum_ps = pp.tile([1, S], f32, name="msum_ps", tag="enc_row", bufs=1)
            for c in range(2):
                nc.tensor.matmul(sum_ps, ones_col, Xc[c], start=(c == 0), stop=(c == 1))
            mrow = wp.tile([1, S], f32, name=f"mrow_{label}")
            nc.scalar.mul(mrow, sum_ps, 1.0 / H)
            m_bc = bcast(pp, mrow, S)
            usq = []
            for c in range(2):
                nc.vector.tensor_tensor(Xc[c], Xc[c], m_bc, op=ALU.subtract)
                u = wp.tile([128, S], f32, name=f"musq_{label}{c}")
                nc.vector.tensor_mul(u, Xc[c], Xc[c])
                usq.append(u)
            return usq

        usqP = center_stats(P, "P")   # P, Q centered in place now
        usqQ = center_stats(Q, "Q")

        varS_col = []
        for blk in range(NB):
            v_ps = pp.tile([128, 1], f32, name="vs_ps", tag="enc_col", bufs=1)
            for c in range(2):
                nc.tensor.matmul(v_ps, usqP[c][:, blk * 128:(blk + 1) * 128],
                                 ones_col, start=(c == 0), stop=(c == 1))
            v = wp.tile([128, 1], f32, name=f"varS{blk}")
            nc.scalar.mul(v, v_ps, 1.0 / H)
            varS_col.append(v)

        vt_ps = pp.tile([1, S], f32, name="vt_ps", tag="enc_row", bufs=1)
        for c in range(2):
            nc.tensor.matmul(vt_ps, ones_col, usqQ[c], start=(c == 0), stop=(c == 1))
        varT_row = wp.tile([1, S], f32)
        nc.scalar.mul(varT_row, vt_ps, 1.0 / H)

        Pg, Qg = [], []
        for c in range(2):
            pg = wp.tile([128, S], f32, name=f"Pg{c}")
            nc.vector.tensor_scalar(pg, P[c], mapg_sb[:, c:c + 1], None, op0=ALU.mult)
            Pg.append(pg)
            qg = wp.tile([128, S], bf16, name=f"Qg{c}")
            nc.vector.tensor_scalar(qg, Q[c], mapg_sb[:, c:c + 1], None, op0=ALU.mult)
            Qg.append(qg)

        # ---------------------------------------------------------------
        # per-block (128 s values): cross-term matmul -> rstd [128, 256],
        # then the big pass. Each matmul's moving tensor packs TWO s values
        # (N=512 = 2x256); matmul outputs must land on psum partitions
        # {0,32,64,96}, so duos are processed in quads (duo 4k+j -> partition
        # 32j of a quad psum tile; the 4 matmuls run concurrently in distinct
        # PE col groups). Quads are drained partition-preserving by DVE/ACT
        # into a sparse SBUF tile, then an SBUF->SBUF DMA scatters rows into
        # the dense [128, 256] score tile:
        #   s_local = 8*k + 2*j + u   (k quad, j col group, u duo half)
        # ---------------------------------------------------------------
        enc_ctx.close()
        scp_ctx = ExitStack()
        scp = scp_ctx.enter_context(tc.tile_pool(name="scp", bufs=2, space="PSUM"))
        act_ctr = 0
        drain_ctr = 0
        rstd_ts = []
        for blk in range(NB):
            dot_ps = scp.tile([128, S], f32, name="dot_ps", tag="dot")
            for c in range(2):
                nc.tensor.matmul(dot_ps, P[c][:, blk * 128:(blk + 1) * 128],
                                 Q[c], start=(c == 0), stop=(c == 1))
            var_t = wp.tile([128, S], f32, name=f"var_t{blk}", tag="var_t", bufs=2)
            nc.vector.tensor_scalar(var_t, dot_ps, 2.0 / H, varS_col[blk],
                                    op0=ALU.mult, op1=ALU.add)
            vt_bc = bcast(scp, varT_row, S, tag="dot")
            nc.vector.tensor_tensor(var_t, var_t, vt_bc, op=ALU.add)
            sq_t = wp.tile([128, S], f32, name=f"sq_t{blk}", tag="sq_t", bufs=2)
            nc.scalar.activation(sq_t, var_t, AF.Sqrt, bias=eps_col)
            rstd_t = wp.tile([128, S], f32, name=f"rstd_t{blk}", tag="rstd_t", bufs=2)
            nc.vector.reciprocal_approx_fast(rstd_t, sq_t)
            rstd_ts.append(rstd_t)

        for blk in range(NB):
            rstd_t = rstd_ts[blk]
            if STAGE < 2:
                nc.sync.dma_start(out_d[blk * 128:(blk + 1) * 128, :], rstd_t)
                continue

            ssb = wp.tile([128, S], f32, name=f"ssb{blk}", tag="ssb", bufs=2)
            for g in range(8):              # 8 groups of 2 quads
                # 2 quads share one 2-bank psum tensor; quad qq fills free
                # half qq. Drained with a single [4, 1024] copy.
                qgps = scp.tile([128, 2, 2, S], f32, name="qgps", tag="quad",
                                bufs=3)
                sparse = wp.tile([128, 2, 2, S], f32, name="sparse",
                                 tag="sparse", bufs=3)
                for qq in range(2):
                    k = 2 * g + qq
                    for c in range(2):
                        zs = []
                        for j in range(4):
                            z = zp.tile([128, 2 * S], bf16, name="z", tag="z")
                            for u in range(2):
                                col = blk * 128 + 8 * k + 2 * j + u
                                zslice = z[:, u * S:(u + 1) * S]
                                if act_ctr % ACT_EVERY == ACT_EVERY - 1:
                                    nc.scalar.activation(zslice, Qg[c], AF.Relu,
                                                         bias=Pg[c][:, col:col + 1])
                                else:
                                    nc.vector.tensor_scalar(
                                        zslice, Qg[c], Pg[c][:, col:col + 1], 0.0,
                                        op0=ALU.add, op1=ALU.max)
                                act_ctr += 1
                            zs.append(z)
                        for j in range(4):
                            nc.tensor.matmul(
                                qgps[32 * j:32 * j + 32, qq, :, :],
                                w2rep[:, c, :], zs[j],
                                start=(c == 0), stop=(c == 1),
                                tile_position=(0, 32 * j),
                                skip_group_check=True)
                # partition-preserving drain of both quads. Only rows {32j}
                # are valid, but compute APs need partition step 1 and DVE
                # cost is free-dim-bound, so copy all 128 rows densely.
                drain_in = qgps.rearrange("p q u t -> p (q u t)")
                drain_out = sparse.rearrange("p q u t -> p (q u t)")
                if drain_ctr % 2 == 0:
                    nc.vector.tensor_copy(drain_out, drain_in)
                else:
                    nc.scalar.copy(drain_out, drain_in)
                drain_ctr += 1
                if STAGE < 3:
                    continue
                # scatter: rows {32j} -> dense ssb rows 8k+2j+u, per (quad, u)
                # rows 8k+2j+u iterate consecutively in (j, u) order, so one
                # DMA per quad writes the contiguous range [8k, 8k+8)
                sp_v = sparse.rearrange("(j x) q u t -> j x q u t", j=4)
                for qq in range(2):
                    k = 2 * g + qq
                    nc.sync.dma_start(ssb[8 * k:8 * k + 8, :],
                                      sp_v[:, 0, qq, :, :])

            if STAGE < 3:
                nc.sync.dma_start(out_d[blk * 128:(blk + 1) * 128, :],
                                  sparse[:, 0, 0, :])
                continue

            # epilogue: rstd multiply + softmax over t (no max-shift needed:
            # |scores| is small by construction, exp cannot overflow)
            nc.vector.tensor_tensor(ssb, ssb, rstd_t, op=ALU.mult)
            omap = wp.tile([128, S], f32, name=f"omap{blk}", tag="omap", bufs=2)
            sume = wp.tile([128, 1], f32, name="sume", tag="smx2", bufs=4)
            nc.scalar.activation(omap, ssb, AF.Exp, accum_out=sume)
            rec = wp.tile([128, 1], f32, name="rec", tag="smx3", bufs=4)
            nc.vector.reciprocal(rec, sume)
            nc.vector.tensor_scalar(omap, omap, rec, None, op0=ALU.mult)
            nc.sync.dma_start(out_d[blk * 128:(blk + 1) * 128, :], omap)

        scp_ctx.close()

        if STAGE < 4:
            return

        # ---------------------------------------------------------------
        # correspondence scorer (tiny): relu(Pe_i + Qe_j) @ w2, softmax
        # ---------------------------------------------------------------
        cp_ctx = ExitStack()
        cp = cp_ctx.enter_context(tc.tile_pool(name="cp", bufs=2, space="PSUM"))
        PeT, QeT = [], []
        for mb in range(2):
            pe_ps = cp.tile([128, N], f32, name="pe_ps", tag="cor_ps")
            for c in range(2):
                nc.tensor.matmul(pe_ps, corW1_sb[:, c, mb * 128:(mb + 1) * 128],
                                 entT["s"][:, c, :], start=(c == 0), stop=(c == 1))
            pe = wp.tile([128, N], f32, name=f"PeT{mb}")
            nc.scalar.copy(pe, pe_ps)
            PeT.append(pe)
            qe_ps = cp.tile([128, N], f32, name="qe_ps", tag="cor_ps")
            for c in range(2):
                nc.tensor.matmul(qe_ps, corW1_sb[:, 2 + c, mb * 128:(mb + 1) * 128],
                                 entT["t"][:, c, :], start=(c == 0), stop=(c == 1))
            qe = wp.tile([128, N], f32, name=f"QeT{mb}")
            nc.scalar.activation(qe, qe_ps, AF.Identity, bias=corb1_sb[:, mb:mb + 1])
            QeT.append(qe)

        cs_ps = cp.tile([1, S], f32, name="cs_ps", tag="cor_row", bufs=1)
        for mb in range(2):
            xr = wp.tile([128, S], f32, name=f"corX{mb}")
            nc.vector.tensor_tensor(
                xr.rearrange("p (i j) -> p i j", i=N),
                QeT[mb].unsqueeze(1).broadcast_to((128, N, N)),
                PeT[mb].unsqueeze(2).broadcast_to((128, N, N)),
                op=ALU.add)
            nc.vector.tensor_scalar(xr, xr, 0.0, None, op0=ALU.max)
            nc.tensor.matmul(cs_ps, corW2_sb[:, mb:mb + 1], xr,
                             start=(mb == 0), stop=(mb == 1))

        cs_row = wp.tile([1, S], f32)
        nc.scalar.copy(cs_row, cs_ps)
        if os.environ.get("K_COR_DEBUG"):
            nc.sync.dma_start(out_d[S:S + 1, :], cs_row)
            cp_ctx.close()
            return
        # bounce through DRAM: partition-expanding SBUF->SBUF DMAs are not
        # expressible as a single regular AP
        dr_ctx = ExitStack()
        drp = dr_ctx.enter_context(tc.tile_pool(name="drp", bufs=1, space="DRAM"))
        cs_dram = drp.tile([1, S], f32, name="cs_dram")
        nc.sync.dma_start(cs_dram, cs_row)
        corr2d = wp.tile([N, N], f32)
        nc.sync.dma_start(corr2d, cs_dram.rearrange("o (i j) -> (o i) j", i=N))
        if os.environ.get("K_COR_DEBUG2"):
            nc.sync.dma_start(out_d[S:S + 1, :].rearrange("o (i j) -> (o i) j", i=N), corr2d)
            cp_ctx.close()
            dr_ctx.close()
            return
        cmax = wp.tile([N, 1], f32)
        nc.vector.tensor_reduce(cmax, corr2d, axis=mybir.AxisListType.X,
                                op=ALU.max, negate=True)
        cex = wp.tile([N, N], f32)
        csum = wp.tile([N, 1], f32)
        nc.scalar.activation(cex, corr2d, AF.Exp, bias=cmax, accum_out=csum)
        crec = wp.tile([N, 1], f32)
        nc.vector.reciprocal(crec, csum)
        nc.vector.tensor_scalar(cex, cex, crec, None, op0=ALU.mult)
        nc.sync.dma_start(out_d[S:S + 1, :].rearrange("o (i j) -> (o i) j", i=N), cex)
        dr_ctx.close()
        cp_ctx.close()


def _declare_io(nc):
    import concourse.mybir as mybir
    f32 = mybir.dt.float32
    shapes = {
        "source_entities": [N, E], "source_relations": [N, N],
        "target_entities": [N, E], "target_relations": [N, N],
        "rel_W1": [2 * E, H], "rel_b1": [H], "rel_g1": [H], "rel_be1": [H],
        "rel_W2": [H, H], "rel_b2": [H],
        "map_W1": [2 * H, H], "map_b1": [H], "map_g": [H],
        "map_W2": [H, 1],
        "cor_W1": [2 * E, H], "cor_b1": [H], "cor_W2": [H, 1],
    }
    io = {k: nc.dram_tensor(k, v, f32, kind="ExternalInput").ap()
          for k, v in shapes.items()}
    io["out"] = nc.dram_tensor("out", [S + 1, S], f32, kind="ExternalOutput").ap()
    return io


def _build():
    import concourse.bacc as bacc
    import concourse.mybir as mybir
    from concourse.tile import TileContext
    nc = bacc.Bacc("TRN2", target_bir_lowering=False, debug=False,
                   enable_asserts=False)
    io = _declare_io(nc)
    with TileContext(nc) as tc:
        _emit(nc, tc, io)
    nc.finalize()
    return nc


def _get_compiled():
    global _COMPILED
    if _COMPILED is None:
        _COMPILED = _build()
    return _COMPILED


def _make_in_maps(inputs):
    names = ["source_entities", "source_relations", "target_entities",
             "target_relations"]
    weights = ["rel_W1", "rel_b1", "rel_g1", "rel_be1", "rel_W2", "rel_b2",
               "map_W1", "map_b1", "map_g", "map_W2", "cor_W1", "cor_b1",
               "cor_W2"]
    wmap = {k: np.ascontiguousarray(np.asarray(inputs[k], np.float32))
            for k in weights}
    in_maps = []
    for i in range(NCORES):
        m = dict(wmap)
        for k in names:
            m[k] = np.ascontiguousarray(np.asarray(inputs[k], np.float32)[i])
        in_maps.append(m)
    return in_maps


def _run_device(inputs, trace=False):
    from concourse.bass_utils import run_bass_kernel_spmd
    nc = _get_compiled()
    res = run_bass_kernel_spmd(nc, _make_in_maps(inputs),
                               core_ids=list(range(NCORES)), trace=trace)
    outs = [np.asarray(r["out"], np.float32) for r in res.results]
    maps = np.stack([o[:S, :] for o in outs]).reshape(B, S, S)
    cors = np.stack([o[S].reshape(N, N) for o in outs])
    return (maps, cors), res


# ---------------------------------------------------------------------------
# numpy fallback (general path; only used if fast-path assumptions break)
# ---------------------------------------------------------------------------

def _ln_np(x, g, b):
    m = x.mean(-1, keepdims=True)
    v = ((x - m) ** 2).mean(-1, keepdims=True)
    return (x - m) / np.sqrt(v + EPS) * g + b


def _softmax_np(x):
    e = np.exp(x - x.max(-1, keepdims=True))
    return e / e.sum(-1, keepdims=True)


def _reference_np(inputs):
    i = {k: np.asarray(v, np.float64) for k, v in inputs.items()}
    n, e = i["source_entities"].shape[1], i["source_entities"].shape[2]

    def enc(ent, rel):
        Wi, Wj = i["rel_W1"][:e], i["rel_W1"][e:]
        h = (np.einsum("bie,eh->bih", ent, Wi)[:, :, None, :]
             + np.einsum("bje,eh->bjh", ent, Wj)[:, None, :, :] + i["rel_b1"])
        h = np.maximum(_ln_np(h, i["rel_g1"], i["rel_be1"]), 0.0)
        h = np.einsum("bijh,hk->bijk", h, i["rel_W2"]) + i["rel_b2"]
        h = h * (rel > 0)[..., None]
        return h.reshape(ent.shape[0], n * n, -1)

    def pair(a, b, W1, b1, W2, b2, g=None, be=None):
        d = a.shape[-1]
        h = (np.einsum("bsd,dh->bsh", a, W1[:d])[:, :, None, :]
             + np.einsum("btd,dh->bth", b, W1[d:])[:, None, :, :] + b1)
        if g is not None:
            h = _ln_np(h, g, be)
        h = np.maximum(h, 0.0)
        return (np.einsum("bsth,ho->bsto", h, W2) + b2)[..., 0]

    sr = enc(i["source_entities"], i["source_relations"])
    tr = enc(i["target_entities"], i["target_relations"])
    ms = pair(sr, tr, i["map_W1"], i["map_b1"], i["map_W2"],
              i.get("map_b2", np.zeros(1)), i["map_g"], i["map_be"])
    cs = pair(i["source_entities"], i["target_entities"], i["cor_W1"],
              i["cor_b1"], i["cor_W2"], i.get("cor_b2", np.zeros(1)))
    return (_softmax_np(ms).astype(np.float32), _softmax_np(cs).astype(np.float32))


def _fast_path_ok(inputs):
    try:
        if tuple(np.asarray(inputs["source_entities"]).shape) != (B, N, E):
            return False
        if not np.all(np.asarray(inputs["map_be"]) == 0.0):
            return False
        for k in ("rel_W1", "rel_W2", "map_W1", "map_W2", "cor_W1", "cor_W2"):
            _ = inputs[k]
        return True
    except Exception:
        return False


def kernel(**inputs):
    if not _fast_path_ok(inputs):
        return _reference_np(inputs)
    (maps, cors), _ = _run_device(inputs)
    return maps, cors


if __name__ == "__main__":
    import reference as ref
    inp = ref.setup_inputs()
    m, c = kernel(**inp)
    print("map", m.shape, m.dtype, "cor", c.shape, c.dtype)


# revision 55
# speedup vs baseline: 1.2319x; 1.2226x over previous
"""Trainium2 Bass kernel for nn_AnalogicalReasoning (gnn_message_passing).

Data-parallel over batch B=8 across the 8 NeuronCores (one batch element per
core). Per core everything is fused on-chip:

  - relation encoder for src/tgt computed in [hidden, pair] layout so that
    LayerNorm reductions over the hidden axis become ones-vector matmuls on
    the TensorEngine (partition-axis reductions are impossible on DVE).
  - the rel-encoder output matmul is fused into the mapping projection: the
    relation mask scales whole relT columns, so it commutes through map_W1
    and P0 = mask o (comb^T Xn + bias) with comb = rel_W2 @ map_W1-half
    computed once on-device (relT is never materialized).
  - the mapping network's LayerNorm over cat([src_rel_s, tgt_rel_t]) @ W1 is
    algebraically separated: x[s,t,:] = P[:,s] + Q[:,t], so mean/var split
    into per-s and per-t statistics plus a cross term (2/H)*Pc^T@Qc computed
    as one matmul per block. With map_be == 0 (checked on host),
       relu(LN(x)) @ w2 = rstd[s,t] * (relu(Pg[:,s] + Qg[:,t]) @ w2)
    which moves the rstd multiply out of the O(S*T*H) inner loop entirely.
    rstd = reciprocal_approx_fast(sqrt(var+eps)) keeps ACT on one table set.
  - the big pass builds Z = max(Qg + p_s, 0) with dual-op tensor_scalar ops
    on DVE (bf16, 2x mode; the per-partition AP scalar caps the mode) with
    every third op routed to ScalarE relu to load-balance. Each matmul packs
    two s-values (N=512) with w2 replicated to M=32 columns; quads of four
    matmuls run concurrently in distinct PE col groups (psum partitions
    {0,32,64,96}), are drained partition-preserving by DVE/ACT into sparse
    SBUF, and scattered to dense score tiles by SBUF->SBUF DMAs.
  - a ~4us PE warmup spin unthrottles the HAM clock gate before the
    latency-bound encoder phase; map_b2/cor_b2 drop (softmax shift
    invariance); softmax needs no max-shift (scores are O(1) by
    construction).

Output per core: [257, 256] f32 — rows 0..255 mapping softmax, row 256 the
flattened 16x16 correspondence softmax.
"""

import numpy as np

B, N, E, H = 8, 16, 256, 256
S = N * N          # 256 source/target relation pairs
EPS = 1e-5
NCORES = 8
NB = S // 128      # 2 s-blocks of 128 per core
ACT_EVERY = 3      # route every 3rd big-pass Z-op to ScalarE

_COMPILED = None


# ---------------------------------------------------------------------------
# graph construction
# ---------------------------------------------------------------------------

def _emit(nc, tc, io):
    import os
    import concourse.mybir as mybir
    STAGE = int(os.environ.get("K_STAGE", "9"))

    f32 = mybir.dt.float32
    bf16 = mybir.dt.bfloat16
    AF = mybir.ActivationFunctionType
    ALU = mybir.AluOpType

    out_d = io["out"]

    from contextlib import ExitStack
    with tc.tile_pool(name="wp", bufs=1) as wp, \
         tc.tile_pool(name="zp", bufs=24) as zp:

        ones_col = wp.tile([128, 1], f32)
        nc.vector.memset(ones_col, 1.0)

        # ---- weights into SBUF, chunked so partition = contraction index ----
        def load_mat(name, rows):
            t = wp.tile([128, rows // 128, io[name].shape[1]], f32, name=f"{name}_sb")
            nc.sync.dma_start(t, io[name].rearrange("(c p) h -> p c h", p=128))
            return t

        def load_vec(name):
            t = wp.tile([128, 2], f32, name=f"{name}_sb")
            nc.sync.dma_start(t, io[name].rearrange("(c p) -> p c", p=128))
            return t

        relW1_sb = load_mat("rel_W1", 2 * E)     # [128, 4, 256]
        relW2_sb = load_mat("rel_W2", H)         # [128, 2, 256]
        mapW1_sb = load_mat("map_W1", 2 * H)     # [128, 4, 256]
        corW1_sb = load_mat("cor_W1", 2 * E)     # [128, 4, 256]
        relb1_sb = load_vec("rel_b1")
        relg1_sb = load_vec("rel_g1")
        relbe1_sb = load_vec("rel_be1")
        relb2_sb = load_vec("rel_b2")
        mapb1_sb = load_vec("map_b1")
        mapg_sb = load_vec("map_g")
        corb1_sb = load_vec("cor_b1")

        mapW2_sb = wp.tile([128, 2], f32)
        nc.sync.dma_start(mapW2_sb, io["map_W2"].rearrange("(c p) o -> p (c o)", p=128))
        w2bf = wp.tile([128, 2], bf16)
        nc.vector.tensor_copy(w2bf, mapW2_sb)
        # w2 replicated to 32 columns: big-pass matmuls write all 32 rows of
        # a col group (M=32) so psum tiles are fully initialized before drain
        w2rep = wp.tile([128, 2, 32], bf16)
        for c in range(2):
            nc.vector.tensor_copy(w2rep[:, c, :],
                                  w2bf[:, c:c + 1].broadcast_to((128, 32)))
        corW2_sb = wp.tile([128, 2], f32)
        nc.sync.dma_start(corW2_sb, io["cor_W2"].rearrange("(c p) o -> p (c o)", p=128))

        # entities transposed [e, i] (2 e-chunks); small strided DMA
        entT = {}
        for side, name in (("s", "source_entities"), ("t", "target_entities")):
            t = wp.tile([128, 2, N], f32, name=f"entT_{side}")
            src = io[name].rearrange("i (c p) -> p c i", p=128)
            for c in range(2):
                nc.sync.dma_start(t[:, c, :], src[:, c, :])
            entT[side] = t

        # relation masks as [1, 256] rows: 1.0 where rel > 0
        mask = {}
        for side, name in (("s", "source_relations"), ("t", "target_relations")):
            raw = wp.tile([1, S], f32, name=f"relraw_{side}")
            nc.sync.dma_start(raw, io[name].rearrange("i j -> (i j)").unsqueeze(0))
            m = wp.tile([1, S], f32, name=f"mask_{side}")
            nc.vector.tensor_scalar(m, raw, 0.0, None, op0=ALU.is_gt)
            mask[side] = m

        ones_row = wp.tile([1, 128], f32)
        nc.vector.memset(ones_row, 1.0)
        twos_row = wp.tile([1, 128], f32)
        nc.vector.memset(twos_row, 2.0)
        eps_col = wp.tile([128, 1], f32)
        nc.vector.memset(eps_col, EPS)

        # PE warmup: ~4us of dense dummy matmuls while input DMAs run, so
        # the HAM clock gate reaches 2.4 GHz before the first real matmul
        # (otherwise the dependency-latency-bound encoder phase runs at
        # 1.2 GHz throughout).
        warm_in = wp.tile([128, 512], bf16)
        nc.vector.memset(warm_in, 0.0)
        warm_w = wp.tile([128, 1], bf16)
        nc.vector.memset(warm_w, 0.0)
        with tc.tile_pool(name="warmp", bufs=1, space="PSUM") as warmp:
            warm_ps = warmp.tile([1, 2 * S], f32)
            for _ in range(16):
                nc.tensor.matmul(warm_ps, warm_w, warm_in, start=True, stop=True)

        # encoder-phase PSUM pool (released before the big pass to stay
        # within the 8 PSUM banks)
        enc_ctx = ExitStack()
        pp = enc_ctx.enter_context(tc.tile_pool(name="pp", bufs=2, space="PSUM"))

        def bcast(pool, row_sb, n, tag="bcast", rows=None):
            # replicate a [1, n] SBUF row across all 128 psum partitions,
            # scaled by the value in `rows` (default ones)
            bp = pool.tile([128, n], f32, name="bc_ps", tag=tag,
                           bufs=2 if tag == "bcast" else None)
            nc.tensor.matmul(bp, rows if rows is not None else ones_row,
                             row_sb, start=True, stop=True)
            return bp

        # ---------------------------------------------------------------
        # relation encoder (per side) -> relT chunks [128, 256] x2 (k, pair)
        # ---------------------------------------------------------------
        def encode(side):
            eT = entT[side]
            # AT/CT: [h, i] = Wi^T @ entT ; Wj rows are chunks 2,3 of rel_W1
            AT, CT = [], []
            for mb in range(2):
                at_ps = pp.tile([128, N], f32, name="at_ps", tag="enc_ps")
                for c in range(2):
                    nc.tensor.matmul(at_ps, relW1_sb[:, c, mb * 128:(mb + 1) * 128],
                                     eT[:, c, :], start=(c == 0), stop=(c == 1))
                a = wp.tile([128, N], f32, name=f"AT{side}{mb}")
                nc.scalar.copy(a, at_ps)
                AT.append(a)
                ct_ps = pp.tile([128, N], f32, name="ct_ps", tag="enc_ps")
                for c in range(2):
                    nc.tensor.matmul(ct_ps, relW1_sb[:, 2 + c, mb * 128:(mb + 1) * 128],
                                     eT[:, c, :], start=(c == 0), stop=(c == 1))
                cc = wp.tile([128, N], f32, name=f"CT{side}{mb}")
                nc.scalar.activation(cc, ct_ps, AF.Identity,
                                     bias=relb1_sb[:, mb:mb + 1])
                CT.append(cc)

            # X[h, i*16+j] = AT[h,i] + CT[h,j]
            X = []
            for mb in range(2):
                x = wp.tile([128, S], f32, name=f"X{side}{mb}")
                nc.vector.tensor_tensor(
                    x.rearrange("p (i j) -> p i j", i=N),
                    CT[mb].unsqueeze(1).broadcast_to((128, N, N)),
                    AT[mb].unsqueeze(2).broadcast_to((128, N, N)),
                    op=ALU.add)
                X.append(x)

            # LayerNorm over h (partition axis, via ones-matmuls)
            sum_ps = pp.tile([1, S], f32, name="sum_ps", tag="enc_row", bufs=1)
            for c in range(2):
                nc.tensor.matmul(sum_ps, ones_col, X[c], start=(c == 0), stop=(c == 1))
            mean_row = wp.tile([1, S], f32, name=f"mean_{side}")
            nc.scalar.mul(mean_row, sum_ps, 1.0 / H)
            mean_bc = bcast(pp, mean_row, S)
            usq = []
            for c in range(2):
                nc.vector.tensor_tensor(X[c], X[c], mean_bc, op=ALU.subtract)
                u = wp.tile([128, S], f32, name=f"usq{side}{c}")
                nc.vector.tensor_mul(u, X[c], X[c])
                usq.append(u)
            var_ps = pp.tile([1, S], f32, name="var_ps", tag="enc_row", bufs=1)
            for c in range(2):
                nc.tensor.matmul(var_ps, ones_col, usq[c], start=(c == 0), stop=(c == 1))
            sqrt_row = wp.tile([1, S], f32, name=f"sqrtr_{side}")
            nc.scalar.activation(sqrt_row, var_ps, AF.Sqrt,
                                 bias=eps_col[0:1, :], scale=1.0 / H)
            rstd_row = wp.tile([1, S], f32, name=f"rstdr_{side}")
            nc.vector.reciprocal_approx_fast(rstd_row, sqrt_row)
            rstd_bc = bcast(pp, rstd_row, S)
            for c in range(2):
                nc.vector.tensor_tensor(X[c], X[c], rstd_bc, op=ALU.mult)
                nc.scalar.activation(X[c], X[c], AF.Relu,
                                     bias=relbe1_sb[:, c:c + 1],
                                     scale=relg1_sb[:, c:c + 1])

            return X

        # ---------------------------------------------------------------
        # fuse rel-encoder output matmul with the mapping projection:
        # the relation mask scales whole relT columns, so it commutes
        # through map_W1:
        #   P0 = map_Wa^T (mask o (rel_W2^T Xn + b2))
        #      = (mask o (comb_a^T Xn + bias_a))  with comb_a = rel_W2 map_Wa
        # Computed once on-device (transpose rel_W2 via PE, 2 matmuls).
        # ---------------------------------------------------------------
        from concourse import masks
        ident = wp.tile([128, 128], f32)
        masks.make_identity(nc, ident)
        relW2T = wp.tile([128, 2, 256], f32)     # [k-part, k-chunk a, h]
        for a in range(2):
            for c in range(2):
                tp_ps = pp.tile([128, 128], f32, name="tp_ps", tag="enc_ops")
                nc.tensor.transpose(tp_ps, relW2_sb[:, c, a * 128:(a + 1) * 128],
                                    ident)
                nc.vector.tensor_copy(relW2T[:, a, c * 128:(c + 1) * 128], tp_ps)
        comb = {}
        biasc = {}
        for which, wch in (("a", 0), ("b", 2)):
            cw = wp.tile([128, 2, 256], f32, name=f"comb{which}")  # [h-part, hb, h']
            for hb in range(2):
                ps = pp.tile([128, S], f32, name="comb_ps", tag="enc_ops")
                for kc in range(2):
                    nc.tensor.matmul(ps, relW2T[:, kc, hb * 128:(hb + 1) * 128],
                                     mapW1_sb[:, wch + kc, :],
                                     start=(kc == 0), stop=(kc == 1))
                nc.vector.tensor_copy(cw[:, hb, :], ps)
            comb[which] = cw
            bc_t = wp.tile([128, 2], f32, name=f"biasc{which}")    # [h'-part, hb]
            for hb in range(2):
                ps1 = pp.tile([128, 1], f32, name="bc1_ps", tag="enc_col", bufs=1)
                for kc in range(2):
                    nc.tensor.matmul(ps1, mapW1_sb[:, wch + kc, hb * 128:(hb + 1) * 128],
                                     relb2_sb[:, kc:kc + 1],
                                     start=(kc == 0), stop=(kc == 1))
                nc.vector.tensor_copy(bc_t[:, hb:hb + 1], ps1)
            biasc[which] = bc_t

        Xs = encode("s")
        Xt = encode("t")

        def project(Xn, which, side, b1_col=None):
            mask_bc = bcast(pp, mask[side], S)
            out = []
            for hb in range(2):
                ps = pp.tile([128, S], f32, name="pq_ps", tag="enc_ops")
                for c in range(2):
                    nc.tensor.matmul(ps, comb[which][:, c, hb * 128:(hb + 1) * 128],
                                     Xn[c], start=(c == 0), stop=(c == 1))
                o = wp.tile([128, S], f32, name=f"proj{which}{hb}")
                nc.scalar.activation(o, ps, AF.Identity,
                                     bias=biasc[which][:, hb:hb + 1])
                nc.vector.tensor_tensor(o, o, mask_bc, op=ALU.mult)
                if b1_col is not None:
                    nc.vector.tensor_scalar(o, o, b1_col[hb], None, op0=ALU.add)
                out.append(o)
            return out

        P = project(Xs, "a", "s")
        Q = project(Xt, "b", "t",
                    [mapb1_sb[:, 0:1], mapb1_sb[:, 1:2]])

        def center_stats(Xc, label):
            sum_ps = pp.tile([1, S], f32, name="msum_ps", tag="enc_row", bufs=1)
            for c in range(2):
                nc.tensor.matmul(sum_ps, ones_col, Xc[c], start=(c == 0), stop=(c == 1))
            mrow = wp.tile([1, S], f32, name=f"mrow_{label}")
            nc.scalar.mul(mrow, sum_ps, 1.0 / H)
            m_bc = bcast(pp, mrow, S)
            usq = []
            for c in range(2):
                nc.vector.tensor_tensor(Xc[c], Xc[c], m_bc, op=ALU.subtract)
                u = wp.tile([128, S], f32, name=f"musq_{label}{c}")
                nc.vector.tensor_mul(u, Xc[c], Xc[c])
                usq.append(u)
            return usq

        usqP = center_stats(P, "P")   # P, Q centered in place now
        usqQ = center_stats(Q, "Q")

        varS_col = []
        for blk in range(NB):
            v_ps = pp.tile([128, 1], f32, name="vs_ps", tag="enc_col", bufs=1)
            for c in range(2):
                nc.tensor.matmul(v_ps, usqP[c][:, blk * 128:(blk + 1) * 128],
                                 ones_col, start=(c == 0), stop=(c == 1))
            v = wp.tile([128, 1], f32, name=f"varS{blk}")
            nc.scalar.mul(v, v_ps, 1.0 / H)
            varS_col.append(v)

        vt_ps = pp.tile([1, S], f32, name="vt_ps", tag="enc_row", bufs=1)
        for c in range(2):
            nc.tensor.matmul(vt_ps, ones_col, usqQ[c], start=(c == 0), stop=(c == 1))
        varT_row = wp.tile([1, S], f32)
        nc.scalar.mul(varT_row, vt_ps, 1.0 / H)

        Pg, Qg = [], []
        for c in range(2):
            pg = wp.tile([128, S], f32, name=f"Pg{c}")
            nc.vector.tensor_scalar(pg, P[c], mapg_sb[:, c:c + 1], None, op0=ALU.mult)
            Pg.append(pg)
            qg = wp.tile([128, S], bf16, name=f"Qg{c}")
            nc.vector.tensor_scalar(qg, Q[c], mapg_sb[:, c:c + 1], None, op0=ALU.mult)
            Qg.append(qg)

        # ---------------------------------------------------------------
        # per-block (128 s values): cross-term matmul -> rstd [128, 256],
        # then the big pass. Each matmul's moving tensor packs TWO s values
        # (N=512 = 2x256); matmul outputs must land on psum partitions
        # {0,32,64,96}, so duos are processed in quads (duo 4k+j -> partition
        # 32j of a quad psum tile; the 4 matmuls run concurrently in distinct
        # PE col groups). Quads are drained partition-preserving by DVE/ACT
        # into a sparse SBUF tile, then an SBUF->SBUF DMA scatters rows into
        # the dense [128, 256] score tile:
        #   s_local = 8*k + 2*j + u   (k quad, j col group, u duo half)
        # ---------------------------------------------------------------
        enc_ctx.close()
        scp_ctx = ExitStack()
        scp = scp_ctx.enter_context(tc.tile_pool(name="scp", bufs=2, space="PSUM"))
        if os.environ.get("K_WARM2", "1") == "1":
            # re-warm the PE clock gate: the stats tail of the encoder phase
            # is row-op heavy and can idle the PE past the HAM window, which
            # would make the first big-pass matmul groups run at 1.2 GHz
            warm2_ps = scp.tile([1, 2 * S], f32, name="warm2_ps", tag="dot")
            for _ in range(8):
                nc.tensor.matmul(warm2_ps, warm_w, warm_in, start=True,
                                 stop=True)
        act_ctr = 0
        drain_ctr = 0
        rstd_ts = []
        for blk in range(NB):
            dot_ps = scp.tile([128, S], f32, name="dot_ps", tag="dot")
            for c in range(2):
                nc.tensor.matmul(dot_ps, P[c][:, blk * 128:(blk + 1) * 128],
                                 Q[c], start=(c == 0), stop=(c == 1))
            var_t = wp.tile([128, S], f32, name=f"var_t{blk}", tag="var_t", bufs=2)
            nc.vector.tensor_scalar(var_t, dot_ps, 2.0 / H, varS_col[blk],
                                    op0=ALU.mult, op1=ALU.add)
            vt_bc = bcast(scp, varT_row, S, tag="dot")
            nc.vector.tensor_tensor(var_t, var_t, vt_bc, op=ALU.add)
            sq_t = wp.tile([128, S], f32, name=f"sq_t{blk}", tag="sq_t", bufs=2)
            nc.scalar.activation(sq_t, var_t, AF.Sqrt, bias=eps_col)
            rstd_t = wp.tile([128, S], f32, name=f"rstd_t{blk}", tag="rstd_t", bufs=2)
            nc.vector.reciprocal_approx_fast(rstd_t, sq_t)
            rstd_ts.append(rstd_t)

        for blk in range(NB):
            rstd_t = rstd_ts[blk]
            if STAGE < 2:
                nc.sync.dma_start(out_d[blk * 128:(blk + 1) * 128, :], rstd_t)
                continue

            ssb = wp.tile([128, S], f32, name=f"ssb{blk}", tag="ssb", bufs=2)
            for g in range(8):              # 8 groups of 2 quads
                # 2 quads share one 2-bank psum tensor; quad qq fills free
                # half qq. Drained with a single [4, 1024] copy.
                qgps = scp.tile([128, 2, 2, S], f32, name="qgps", tag="quad",
                                bufs=3)
                sparse = wp.tile([128, 2, 2, S], f32, name="sparse",
                                 tag="sparse", bufs=3)
                for qq in range(2):
                    k = 2 * g + qq
                    for c in range(2):
                        zs = []
                        for j in range(4):
                            z = zp.tile([128, 2 * S], bf16, name="z", tag="z")
                            for u in range(2):
                                col = blk * 128 + 8 * k + 2 * j + u
                                zslice = z[:, u * S:(u + 1) * S]
                                if act_ctr % ACT_EVERY == ACT_EVERY - 1:
                                    nc.scalar.activation(zslice, Qg[c], AF.Relu,
                                                         bias=Pg[c][:, col:col + 1])
                                else:
                                    nc.vector.tensor_scalar(
                                        zslice, Qg[c], Pg[c][:, col:col + 1], 0.0,
                                        op0=ALU.add, op1=ALU.max)
                                act_ctr += 1
                            zs.append(z)
                        for j in range(4):
                            nc.tensor.matmul(
                                qgps[32 * j:32 * j + 32, qq, :, :],
                                w2rep[:, c, :], zs[j],
                                start=(c == 0), stop=(c == 1),
                                tile_position=(0, 32 * j),
                                skip_group_check=True)
                # partition-preserving drain of both quads. Only rows {32j}
                # are valid, but compute APs need partition step 1 and DVE
                # cost is free-dim-bound, so copy all 128 rows densely.
                drain_in = qgps.rearrange("p q u t -> p (q u t)")
                drain_out = sparse.rearrange("p q u t -> p (q u t)")
                if drain_ctr % 2 == 0:
                    nc.vector.tensor_copy(drain_out, drain_in)
                else:
                    nc.scalar.copy(drain_out, drain_in)
                drain_ctr += 1
                if STAGE < 3:
                    continue
                # scatter: rows {32j} -> dense ssb rows 8k+2j+u, per (quad, u)
                # rows 8k+2j+u iterate consecutively in (j, u) order, so one
                # DMA per quad writes the contiguous range [8k, 8k+8)
                sp_v = sparse.rearrange("(j x) q u t -> j x q u t", j=4)
                for qq in range(2):
                    k = 2 * g + qq
                    nc.sync.dma_start(ssb[8 * k:8 * k + 8, :],
                                      sp_v[:, 0, qq, :, :])

            if STAGE < 3:
                nc.sync.dma_start(out_d[blk * 128:(blk + 1) * 128, :],
                                  sparse[:, 0, 0, :])
                continue

            # epilogue: rstd multiply + softmax over t (no max-shift needed:
            # |scores| is small by construction, exp cannot overflow)
            nc.vector.tensor_tensor(ssb, ssb, rstd_t, op=ALU.mult)
            omap = wp.tile([128, S], f32, name=f"omap{blk}", tag="omap", bufs=2)
            sume = wp.tile([128, 1], f32, name="sume", tag="smx2", bufs=4)
            nc.scalar.activation(omap, ssb, AF.Exp, accum_out=sume)
            rec = wp.tile([128, 1], f32, name="rec", tag="smx3", bufs=4)
            nc.vector.reciprocal(rec, sume)
            nc.vector.tensor_scalar(omap, omap, rec, None, op0=ALU.mult)
            nc.sync.dma_start(out_d[blk * 128:(blk + 1) * 128, :], omap)

        scp_ctx.close()

        if STAGE < 4:
            return

        # ---------------------------------------------------------------
        # correspondence scorer (tiny): relu(Pe_i + Qe_j) @ w2, softmax
        # ---------------------------------------------------------------
        cp_ctx = ExitStack()
        cp = cp_ctx.enter_context(tc.tile_pool(name="cp", bufs=2, space="PSUM"))
        PeT, QeT = [], []
        for mb in range(2):
            pe_ps = cp.tile([128, N], f32, name="pe_ps", tag="cor_ps")
            for c in range(2):
                nc.tensor.matmul(pe_ps, corW1_sb[:, c, mb * 128:(mb + 1) * 128],
                                 entT["s"][:, c, :], start=(c == 0), stop=(c == 1))
            pe = wp.tile([128, N], f32, name=f"PeT{mb}")
            nc.scalar.copy(pe, pe_ps)
            PeT.append(pe)
            qe_ps = cp.tile([128, N], f32, name="qe_ps", tag="cor_ps")
            for c in range(2):
                nc.tensor.matmul(qe_ps, corW1_sb[:, 2 + c, mb * 128:(mb + 1) * 128],
                                 entT["t"][:, c, :], start=(c == 0), stop=(c == 1))
            qe = wp.tile([128, N], f32, name=f"QeT{mb}")
            nc.scalar.activation(qe, qe_ps, AF.Identity, bias=corb1_sb[:, mb:mb + 1])
            QeT.append(qe)

        cs_ps = cp.tile([1, S], f32, name="cs_ps", tag="cor_row", bufs=1)
        for mb in range(2):
            xr = wp.tile([128, S], f32, name=f"corX{mb}")
            nc.vector.tensor_tensor(
                xr.rearrange("p (i j) -> p i j", i=N),
                QeT[mb].unsqueeze(1).broadcast_to((128, N, N)),
                PeT[mb].unsqueeze(2).broadcast_to((128, N, N)),
                op=ALU.add)
            nc.vector.tensor_scalar(xr, xr, 0.0, None, op0=ALU.max)
            nc.tensor.matmul(cs_ps, corW2_sb[:, mb:mb + 1], xr,
                             start=(mb == 0), stop=(mb == 1))

        cs_row = wp.tile([1, S], f32)
        nc.scalar.copy(cs_row, cs_ps)
        if os.environ.get("K_COR_DEBUG"):
            nc.sync.dma_start(out_d[S:S + 1, :], cs_row)
            cp_ctx.close()
            return
        # bounce through DRAM: partition-expanding SBUF->SBUF DMAs are not
        # expressible as a single regular AP
        dr_ctx = ExitStack()
        drp = dr_ctx.enter_context(tc.tile_pool(name="drp", bufs=1, space="DRAM"))
        cs_dram = drp.tile([1, S], f32, name="cs_dram")
        nc.sync.dma_start(cs_dram, cs_row)
        corr2d = wp.tile([N, N], f32)
        nc.sync.dma_start(corr2d, cs_dram.rearrange("o (i j) -> (o i) j", i=N))
        if os.environ.get("K_COR_DEBUG2"):
            nc.sync.dma_start(out_d[S:S + 1, :].rearrange("o (i j) -> (o i) j", i=N), corr2d)
            cp_ctx.close()
            dr_ctx.close()
            return
        cmax = wp.tile([N, 1], f32)
        nc.vector.tensor_reduce(cmax, corr2d, axis=mybir.AxisListType.X,
                                op=ALU.max, negate=True)
        cex = wp.tile([N, N], f32)
        csum = wp.tile([N, 1], f32)
        nc.scalar.activation(cex, corr2d, AF.Exp, bias=cmax, accum_out=csum)
        crec = wp.tile([N, 1], f32)
        nc.vector.reciprocal(crec, csum)
        nc.vector.tensor_scalar(cex, cex, crec, None, op0=ALU.mult)
        nc.sync.dma_start(out_d[S:S + 1, :].rearrange("o (i j) -> (o i) j", i=N), cex)
        dr_ctx.close()
        cp_ctx.close()


def _declare_io(nc):
    import concourse.mybir as mybir
    f32 = mybir.dt.float32
    shapes = {
        "source_entities": [N, E], "source_relations": [N, N],
        "target_entities": [N, E], "target_relations": [N, N],
        "rel_W1": [2 * E, H], "rel_b1": [H], "rel_g1": [H], "rel_be1": [H],
        "rel_W2": [H, H], "rel_b2": [H],
        "map_W1": [2 * H, H], "map_b1": [H], "map_g": [H],
        "map_W2": [H, 1],
        "cor_W1": [2 * E, H], "cor_b1": [H], "cor_W2": [H, 1],
    }
    io = {k: nc.dram_tensor(k, v, f32, kind="ExternalInput").ap()
          for k, v in shapes.items()}
    io["out"] = nc.dram_tensor("out", [S + 1, S], f32, kind="ExternalOutput").ap()
    return io


def _build():
    import concourse.bacc as bacc
    import concourse.mybir as mybir
    from concourse.tile import TileContext
    nc = bacc.Bacc("TRN2", target_bir_lowering=False, debug=False,
                   enable_asserts=False)
    io = _declare_io(nc)
    with TileContext(nc) as tc:
        _emit(nc, tc, io)
    nc.finalize()
    return nc


def _get_compiled():
    global _COMPILED
    if _COMPILED is None:
        _COMPILED = _build()
    return _COMPILED


def _make_in_maps(inputs):
    names = ["source_entities", "source_relations", "target_entities",
             "target_relations"]
    weights = ["rel_W1", "rel_b1", "rel_g1", "rel_be1", "rel_W2", "rel_b2",
               "map_W1", "map_b1", "map_g", "map_W2", "cor_W1", "cor_b1",
               "cor_W2"]
    wmap = {k: np.ascontiguousarray(np.asarray(inputs[k], np.float32))
            for k in weights}
    in_maps = []
    for i in range(NCORES):
        m = dict(wmap)
        for k in names:
            m[k] = np.ascontiguousarray(np.asarray(inputs[k], np.float32)[i])
        in_maps.append(m)
    return in_maps


def _run_device(inputs, trace=False):
    from concourse.bass_utils import run_bass_kernel_spmd
    nc = _get_compiled()
    res = run_bass_kernel_spmd(nc, _make_in_maps(inputs),
                               core_ids=list(range(NCORES)), trace=trace)
    outs = [np.asarray(r["out"], np.float32) for r in res.results]
    maps = np.stack([o[:S, :] for o in outs]).reshape(B, S, S)
    cors = np.stack([o[S].reshape(N, N) for o in outs])
    return (maps, cors), res


# ---------------------------------------------------------------------------
# numpy fallback (general path; only used if fast-path assumptions break)
# ---------------------------------------------------------------------------

def _ln_np(x, g, b):
    m = x.mean(-1, keepdims=True)
    v = ((x - m) ** 2).mean(-1, keepdims=True)
    return (x - m) / np.sqrt(v + EPS) * g + b


def _softmax_np(x):
    e = np.exp(x - x.max(-1, keepdims=True))
    return e / e.sum(-1, keepdims=True)


def _reference_np(inputs):
    i = {k: np.asarray(v, np.float64) for k, v in inputs.items()}
    n, e = i["source_entities"].shape[1], i["source_entities"].shape[2]

    def enc(ent, rel):
        Wi, Wj = i["rel_W1"][:e], i["rel_W1"][e:]
        h = (np.einsum("bie,eh->bih", ent, Wi)[:, :, None, :]
             + np.einsum("bje,eh->bjh", ent, Wj)[:, None, :, :] + i["rel_b1"])
        h = np.maximum(_ln_np(h, i["rel_g1"], i["rel_be1"]), 0.0)
        h = np.einsum("bijh,hk->bijk", h, i["rel_W2"]) + i["rel_b2"]
        h = h * (rel > 0)[..., None]
        return h.reshape(ent.shape[0], n * n, -1)

    def pair(a, b, W1, b1, W2, b2, g=None, be=None):
        d = a.shape[-1]
        h = (np.einsum("bsd,dh->bsh", a, W1[:d])[:, :, None, :]
             + np.einsum("btd,dh->bth", b, W1[d:])[:, None, :, :] + b1)
        if g is not None:
            h = _ln_np(h, g, be)
        h = np.maximum(h, 0.0)
        return (np.einsum("bsth,ho->bsto", h, W2) + b2)[..., 0]

    sr = enc(i["source_entities"], i["source_relations"])
    tr = enc(i["target_entities"], i["target_relations"])
    ms = pair(sr, tr, i["map_W1"], i["map_b1"], i["map_W2"],
              i.get("map_b2", np.zeros(1)), i["map_g"], i["map_be"])
    cs = pair(i["source_entities"], i["target_entities"], i["cor_W1"],
              i["cor_b1"], i["cor_W2"], i.get("cor_b2", np.zeros(1)))
    return (_softmax_np(ms).astype(np.float32), _softmax_np(cs).astype(np.float32))


def _fast_path_ok(inputs):
    try:
        if tuple(np.asarray(inputs["source_entities"]).shape) != (B, N, E):
            return False
        if not np.all(np.asarray(inputs["map_be"]) == 0.0):
            return False
        for k in ("rel_W1", "rel_W2", "map_W1", "map_W2", "cor_W1", "cor_W2"):
            _ = inputs[k]
        return True
    except Exception:
        return False


def kernel(**inputs):
    if not _fast_path_ok(inputs):
        return _reference_np(inputs)
    (maps, cors), _ = _run_device(inputs)
    return maps, cors


if __name__ == "__main__":
    import reference as ref
    inp = ref.setup_inputs()
    m, c = kernel(**inp)
    print("map", m.shape, m.dtype, "cor", c.shape, c.dtype)
